# revision 53
# baseline (speedup 1.0000x reference)
"""Trainium2 Bass kernel for nn_AttentionModule (cross-attention pooling).

Reference computation (B=32, T=2048, H=1024):
    score[b,t] = sum_o (enc[b,t,:] @ W[o,:]) * hidden[b,o]
    attn       = softmax(score, axis=t)
    context[b] = sum_t attn[b,t] * enc[b,t,:]

Algebraic rewrite: score[b,t] = enc[b,t,:] . q[b]  with  q = hidden @ W.
This avoids the [B,T,H]x[H,H] matmul entirely and makes the problem
memory-bound: encoder_outputs (256 MB) is read exactly once from HBM.

Sharding: data-parallel over batch, 4 batches per core on 8 cores; W
replicated. Per core per batch: encoder tiles [128t, 1024h] stay resident
in SBUF (8 MB/batch, ~1.75 batches in flight), so the score pass (DVE fused
multiply+accumulate against broadcast q) and the context pass (PE matmuls
contracting over t) share one DMA read.
"""

import numpy as np

B, T, H = 32, 2048, 1024
NCORES = 8
BPC = B // NCORES  # batches per core
NT = T // 128      # 16 row-tiles of 128 per batch
HC = H // 128      # 8 column-chunks of 128

_CACHE = {}


def _split_multi_waits(nc, mybir):
    """Walrus in this env encodes at most 1 sync-wait per instruction for
    several encodings (Drain, LDWEIGHTS, ...). Split extra waits onto
    standalone EventSemaphore instrs placed just before, on the same engine
    (engines are sequential, so ordering is preserved)."""
    for fn in nc.m.functions:
        for blk in fn.blocks:
            new_insts = []
            for inst in blk.instructions:
                si = inst.sync_info
                if si is not None and len(si.on_wait) > 1:
                    waits = list(si.on_wait)
                    for j, w in enumerate(waits[:-1]):
                        new_insts.append(
                            mybir.InstEventSemaphore(
                                name=f"{inst.name}-wsplit{j}",
                                engine=inst.engine,
                                sync_info=mybir.SyncInfo(on_wait=[w], on_update=[]),
                            )
                        )
                    inst.sync_info = mybir.SyncInfo(
                        on_wait=[waits[-1]], on_update=list(si.on_update)
                    )
                new_insts.append(inst)
            blk.instructions = new_insts


def build_nc(split_waits=True):
    import concourse.bass as bass
    import concourse.tile as tile
    from concourse import mybir
    from concourse.masks import make_identity

    f32 = mybir.dt.float32
    f32r_ = mybir.dt.float32r
    nc = bass.Bass(trn_type="TRN2")

    hid = nc.dram_tensor("hidden", [BPC, H], f32, kind="ExternalInput")
    enc = nc.dram_tensor("enc", [BPC, T, H], f32r_, kind="ExternalInput")
    Wt = nc.dram_tensor("W", [H, H], f32, kind="ExternalInput")
    ctx_out = nc.dram_tensor("ctx", [BPC, H], f32, kind="ExternalOutput")
    attn_out = nc.dram_tensor("attn", [BPC, T], f32, kind="ExternalOutput")

    with tile.TileContext(nc) as tc:
        with (
            tc.tile_pool(name="singles", bufs=1) as singles,
            tc.tile_pool(name="qb", bufs=BPC) as qb_pool,
            tc.tile_pool(name="enc", bufs=7) as enc_pool,
            tc.tile_pool(name="wpool", bufs=2 * HC) as wpool,
            tc.tile_pool(name="scr", bufs=3) as scr_pool,
            tc.tile_pool(name="score", bufs=2) as score_pool,
            tc.tile_pool(name="sm", bufs=2) as sm_pool,
            tc.tile_pool(name="psum_big", bufs=2, space="PSUM") as psum_big,
            tc.tile_pool(name="psum_small", bufs=3, space="PSUM") as psum_small,
            tc.tile_pool(name="psum_warm", bufs=1, space="PSUM") as psum_warm,
        ):
            identity = singles.tile([128, 128], f32)
            make_identity(nc, identity)
            ones = singles.tile([128, 128], f32)
            nc.vector.memset(ones, 1.0)

            # hidden transposed with batch b at column 32*b: the lhsT
            # column index IS the PSUM output partition, so q rows land on
            # partitions {0,32,64,96} — all legal matmul rhs base partitions.
            # Columns in between are never read (their PSUM rows are unused).
            M97 = 32 * (BPC - 1) + 1
            hT = singles.tile([128, HC, M97], f32)
            nc.gpsimd.memset(hT, 0.0)
            # PE warm-up: ~5us of dummy matmuls so the HAM clock-gate opens
            # before the q chain starts (cold PE runs fp32 matmuls at half
            # speed; the gate needs ~3.4us of sustained activity).
            def pe_warm(n):
                for _ in range(n):
                    warm_ps = psum_warm.tile([128, 128], f32, tag="warm")
                    nc.tensor.matmul(
                        warm_ps, lhsT=ones, rhs=ones, start=True, stop=True
                    )

            pe_warm(14)

            hre = hid.rearrange("b (c p) -> p c b", p=128)  # [128, HC, BPC]
            hslc = hT[:, 0, 0:1]
            el = hslc.ap[-1][0]
            for b in range(BPC):
                dst = bass.AP(
                    tensor=hslc.tensor,
                    offset=hslc.offset + 32 * b * el,
                    ap=[list(hslc.ap[0]), [M97 * el, HC]],
                )
                nc.sync.dma_start(out=dst, in_=hre[:, :, b])

            # W loaded as 16 half-chunks so the q matmul chain pipelines
            # with W DMA arrival instead of waiting for all 4 MB.
            w_half = [[None] * 2 for _ in range(HC)]
            for c in range(HC):
                for hh in range(2):
                    wtile = wpool.tile([128, 512], f32, tag="w")
                    nc.sync.dma_start(
                        out=wtile,
                        in_=Wt[c * 128 : (c + 1) * 128, hh * 512 : hh * 512 + 512],
                    )
                    w_half[c][hh] = wtile

            # q = hidden @ W in one matmul chain; with the spread hT the
            # q rows land on PSUM partitions {0,32,64,96}
            q_ps = psum_big.tile([M97, H], f32, tag="big")
            for c in range(HC):
                for hh in range(2):
                    nc.tensor.matmul(
                        q_ps[:, hh * 512 : hh * 512 + 512],
                        lhsT=hT[:, c, :],
                        rhs=w_half[c][hh],
                        start=(c == 0),
                        stop=(c == HC - 1),
                    )
            q_sb = singles.tile([M97, H], f32)
            for b in range(BPC):
                nc.scalar.copy(
                    q_sb[32 * b : 32 * b + 1, :], q_ps[32 * b : 32 * b + 1, :]
                )

            # broadcast q[b] across 128 partitions via ones-matmul reading
            # directly from partition 32b (no DMA on the q critical path)
            q_bcast = []
            for b in range(BPC):
                qb = qb_pool.tile([128, H], f32, tag="qb")
                for h in range(0, H, 512):
                    ps = psum_small.tile([128, 512], f32, tag="small")
                    nc.tensor.matmul(
                        ps,
                        lhsT=ones[32 * b : 32 * b + 1, :],
                        rhs=q_sb[32 * b : 32 * b + 1, h : h + 512],
                        start=True, stop=True,
                        tile_position=(32 * b, 0),
                    )
                    nc.scalar.copy(qb[:, h : h + 512], ps)
                q_bcast.append(qb)

            f32r = mybir.dt.float32r
            Exp = mybir.ActivationFunctionType.Exp
            NCH = 4                    # row-tiles per DMA chunk (2 MB)
            HNT = NT // 2              # tiles per half-batch
            for b in range(BPC):
                # Half-batch "flash" structure: each half gets its own local
                # max, exp, running sum and PSUM context accumulation; the
                # two halves are combined exactly at the end. This lets the
                # context matmuls of the first half (and its enc slots)
                # retire while the second half is still streaming in.
                score = score_pool.tile([128, NT], f32, tag="score")
                wexp = score_pool.tile([128, NT], f32r_, tag="wexp")
                negm2 = sm_pool.tile([1, 2], f32, tag="negm2")
                s2 = sm_pool.tile([1, 2], f32, tag="s2")
                enc_tiles = []
                ctx_halves = []
                for half in range(2):
                    sl = slice(half * HNT, (half + 1) * HNT)
                    # ---- load enc chunks + fused mult+reduce -> scores
                    chunk_sizes = (
                        [4, 2, 2] if (b == BPC - 1 and half == 1) else [4, 4]
                    )
                    t0h = half * HNT
                    for csz in chunk_sizes:
                        ch = enc_pool.tile([128, NCH, H], f32r_, tag="enc")
                        nc.sync.dma_start(
                            out=ch[:, 0:csz, :],
                            in_=enc[
                                b, t0h * 128 : (t0h + csz) * 128, :
                            ].rearrange("(j p) h -> p j h", p=128),
                        )
                        for j in range(csz):
                            i = t0h + j
                            et = ch[:, j, :]
                            enc_tiles.append(et)
                            scr = scr_pool.tile([128, H], f32, tag="scr")
                            nc.vector.scalar_tensor_tensor(
                                out=scr,
                                in0=et.bitcast(f32),
                                scalar=1.0,
                                in1=q_bcast[b],
                                op0=mybir.AluOpType.mult,
                                op1=mybir.AluOpType.mult,
                                accum_out=score[:, i : i + 1],
                            )
                        t0h += csz

                    # ---- local softmax for this half (layout [128p, 8i])
                    m_p = sm_pool.tile([128, 1], f32, tag="mp")
                    nc.vector.tensor_reduce(
                        m_p, score[:, sl], axis=mybir.AxisListType.X,
                        op=mybir.AluOpType.max,
                    )
                    tp = psum_small.tile([1, 128], f32, tag="small")
                    nc.tensor.transpose(tp, m_p, identity)
                    nc.vector.tensor_reduce(
                        negm2[0:1, half : half + 1], tp,
                        axis=mybir.AxisListType.X,
                        op=mybir.AluOpType.max, negate=True,
                    )
                    nb_ps = psum_small.tile([128, 1], f32, tag="small")
                    nc.tensor.matmul(
                        nb_ps, lhsT=ones[0:1, :], rhs=negm2[0:1, half : half + 1],
                        start=True, stop=True,
                    )
                    negb = sm_pool.tile([128, 1], f32, tag="negb")
                    nc.vector.tensor_copy(negb, nb_ps)
                    esum = sm_pool.tile([128, 1], f32, tag="esum")
                    nc.scalar.activation(
                        wexp[:, sl], score[:, sl], func=Exp,
                        bias=negb, accum_out=esum,
                    )
                    tp2 = psum_small.tile([1, 128], f32, tag="small")
                    nc.tensor.transpose(tp2, esum, identity)
                    nc.vector.tensor_reduce(
                        s2[0:1, half : half + 1], tp2,
                        axis=mybir.AxisListType.X, op=mybir.AluOpType.add,
                    )

                    # ---- context accumulation (float32r streams at
                    # 1 cyc/col; fp32 pays 4x; the truncation is harmless
                    # for a weighted average). Half B's matmuls are emitted
                    # AFTER the combine scalars below so the scheduler can
                    # interleave the combine/attn ops with the final burst.
                    if half == 0:
                        ctx_ps = psum_big.tile([1, H], f32, tag="big")
                        for i in range(HNT):
                            for h in (0, 512):
                                nc.tensor.matmul(
                                    ctx_ps[0:1, h : h + 512],
                                    lhsT=wexp[:, i : i + 1],
                                    rhs=enc_tiles[i][:, h : h + 512],
                                    start=(i == 0),
                                    stop=(i == HNT - 1),
                                )
                        ctx_halves.append(ctx_ps)

                # ---- combine halves exactly (no Ln — use exact
                # reciprocal so attention weights stay fp32-accurate):
                #   M = max(gA, gB); f_h = exp(g_h - M); Z = SA*fA + SB*fB
                #   attn = exp(score - M) / Z; ctx = (ctxA*fA + ctxB*fB)/Z
                negM = sm_pool.tile([1, 1], f32, tag="negM")
                nc.vector.tensor_reduce(
                    negM, negm2, axis=mybir.AxisListType.X, op=mybir.AluOpType.min
                )
                dAB = sm_pool.tile([1, 2], f32, tag="dAB")  # = M - g_h >= 0
                nc.vector.tensor_scalar(
                    dAB, negm2, negM[0:1, 0:1], None,
                    op0=mybir.AluOpType.subtract,
                )
                eAB = sm_pool.tile([1, 2], f32, tag="eAB")  # f_h = exp(g_h - M)
                nc.scalar.activation(eAB, dAB, func=Exp, scale=-1.0)
                zAB = sm_pool.tile([1, 2], f32, tag="zAB")
                nc.vector.tensor_mul(zAB, s2, eAB)
                Zt = sm_pool.tile([1, 1], f32, tag="Zt")
                nc.vector.tensor_reduce(
                    Zt, zAB, axis=mybir.AxisListType.X, op=mybir.AluOpType.add
                )
                rZ = sm_pool.tile([1, 1], f32, tag="rZ")
                nc.vector.reciprocal(rZ, Zt)
                fr = sm_pool.tile([1, 2], f32, tag="fr")  # f_h / Z
                nc.vector.tensor_scalar_mul(fr, eAB, rZ[0:1, 0:1])

                # scale half-A context now (only needs fr) so ACT runs it
                # during the half-B context burst below
                t1 = sm_pool.tile([1, H], f32, tag="t1")
                nc.scalar.mul(t1, ctx_halves[0], fr[0:1, 0:1])

                # normalized attention weights: exp(score - M) * (1/Z),
                # all in exact fp32 (independent of the f32r context path)
                nb2_ps = psum_small.tile([128, 1], f32, tag="small")
                nc.tensor.matmul(
                    nb2_ps, lhsT=ones[0:1, :], rhs=negM, start=True, stop=True
                )
                negbM = sm_pool.tile([128, 1], f32, tag="negbM")
                nc.vector.tensor_copy(negbM, nb2_ps)
                wg = score_pool.tile([128, NT], f32, tag="wg")
                nc.scalar.activation(wg, score, func=Exp, bias=negbM)
                rp_ps = psum_small.tile([128, 1], f32, tag="small")
                nc.tensor.matmul(rp_ps, lhsT=ones[0:1, :], rhs=rZ, start=True, stop=True)
                rp = sm_pool.tile([128, 1], f32, tag="rp")
                nc.vector.tensor_copy(rp, rp_ps)
                wn = score_pool.tile([128, NT], f32, tag="wn")
                nc.vector.tensor_scalar_mul(wn, wg, rp)

                # attn out: transpose [128,16] -> [16,128] so the DRAM write
                # is contiguous (t = i*128 + p)
                wt_ps = psum_small.tile([16, 128], f32, tag="small")
                nc.tensor.transpose(wt_ps, wn, identity)
                wn_t = sm_pool.tile([16, 128], f32, tag="wnt")
                nc.vector.tensor_copy(wn_t, wt_ps)
                nc.sync.dma_start(
                    out=attn_out[b].rearrange("(i p) -> i p", p=128), in_=wn_t
                )

                # half-B context burst (emitted after the combine/attn ops
                # so their engine-stream order lets them overlap this burst)
                ctxB_ps = psum_big.tile([1, H], f32, tag="big")
                for i in range(HNT, NT):
                    for h in (0, 512):
                        nc.tensor.matmul(
                            ctxB_ps[0:1, h : h + 512],
                            lhsT=wexp[:, i : i + 1],
                            rhs=enc_tiles[i][:, h : h + 512],
                            start=(i == HNT),
                            stop=(i == NT - 1),
                        )
                ctx_halves.append(ctxB_ps)

                # ctx = t1 + ctxB*(fB/Z): one fused op after the last matmul
                ctx_sb = sm_pool.tile([1, H], f32, tag="ctxsb")
                nc.vector.scalar_tensor_tensor(
                    out=ctx_sb,
                    in0=ctx_halves[1],
                    scalar=fr[0:1, 1:2],
                    in1=t1,
                    op0=mybir.AluOpType.mult,
                    op1=mybir.AluOpType.add,
                )
                nc.sync.dma_start(out=ctx_out[b : b + 1, :], in_=ctx_sb)

    if split_waits:
        _split_multi_waits(nc, mybir)
    return nc


def _get_runner():
    """Build (once) a cached jitted shard_map callable running the Bass NEFF
    on 8 cores. Mirrors concourse.bass2jax.run_bass_via_pjrt, but reusable
    across calls so the NEFF compiles exactly once per process."""
    if "runner" in _CACHE:
        return _CACHE["runner"]

    import jax
    from jax.experimental.shard_map import shard_map
    from jax.sharding import Mesh, PartitionSpec
    from concourse import mybir
    from concourse.bass2jax import (
        _bass_exec_p,
        install_neuronx_cc_hook,
        partition_id_tensor,
    )

    nc = build_nc()
    install_neuronx_cc_hook()

    partition_name = nc.partition_id_tensor.name if nc.partition_id_tensor else None
    in_names, out_names, out_avals, zero_outs = [], [], [], []
    for alloc in nc.m.functions[0].allocations:
        if not isinstance(alloc, mybir.MemoryLocationSet):
            continue
        name = alloc.memorylocations[0].name
        if alloc.kind == "ExternalInput":
            if name != partition_name:
                in_names.append(name)
        elif alloc.kind == "ExternalOutput":
            out_names.append(name)
            shape = tuple(alloc.tensor_shape)
            dtype = mybir.dt.np(alloc.dtype)
            out_avals.append(jax.core.ShapedArray(shape, dtype))
            zero_outs.append(np.zeros(shape, dtype))
    n_params = len(in_names)
    param_names = list(in_names)
    in_names = in_names + out_names
    if partition_name is not None:
        in_names.append(partition_name)
    donate = tuple(range(n_params, n_params + len(out_names)))

    def _body(*args):
        operands = list(args)
        if partition_name is not None:
            operands.append(partition_id_tensor())
        return tuple(
            _bass_exec_p.bind(
                *operands,
                out_avals=tuple(out_avals),
                in_names=tuple(in_names),
                out_names=tuple(out_names),
                lowering_input_output_aliases=(),
                sim_require_finite=True,
                sim_require_nnan=True,
                nc=nc,
            )
        )

    devices = jax.devices()[:NCORES]
    mesh = Mesh(np.asarray(devices), ("core",))
    in_specs = (PartitionSpec("core"),) * (n_params + len(out_names))
    out_specs = (PartitionSpec("core"),) * len(out_names)
    sharded = jax.jit(
        shard_map(
            _body, mesh=mesh, in_specs=in_specs, out_specs=out_specs,
            check_rep=False,
        ),
        donate_argnums=donate,
        keep_unused=True,
    )
    runner = {
        "fn": sharded,
        "param_names": param_names,
        "out_names": out_names,
        "out_avals": out_avals,
        "mesh": mesh,
    }
    _CACHE["runner"] = runner
    return runner


def _pack_inputs(hidden, encoder_outputs, W):
    """Concatenate per-core shards along axis 0 (shard_map splits axis 0)."""
    hidden = np.ascontiguousarray(hidden, dtype=np.float32)
    enc = np.ascontiguousarray(encoder_outputs, dtype=np.float32)
    W = np.ascontiguousarray(W, dtype=np.float32)
    full = {
        "hidden": hidden,                      # [32, H] -> 8 x [4, H]
        "enc": enc,                            # [32, T, H] -> 8 x [4, T, H]
        "W": np.concatenate([W] * NCORES, 0),  # replicated -> [8*H, H]
    }
    return full


def _fresh_zeros():
    r = _get_runner()
    return [
        np.zeros((NCORES * a.shape[0], *a.shape[1:]), a.dtype)
        for a in r["out_avals"]
    ]


def kernel(hidden, encoder_outputs, W):
    r = _get_runner()
    full = _pack_inputs(hidden, encoder_outputs, W)
    args = [full[name] for name in r["param_names"]]
    outs = r["fn"](*args, *_fresh_zeros())
    outd = {name: np.asarray(o) for name, o in zip(r["out_names"], outs)}
    return outd["ctx"], outd["attn"]


# revision 60
# speedup vs baseline: 1.0008x; 1.0008x over previous
"""Trainium2 Bass kernel for nn_AttentionModule (cross-attention pooling).

Reference computation (B=32, T=2048, H=1024):
    score[b,t] = sum_o (enc[b,t,:] @ W[o,:]) * hidden[b,o]
    attn       = softmax(score, axis=t)
    context[b] = sum_t attn[b,t] * enc[b,t,:]

Algebraic rewrite: score[b,t] = enc[b,t,:] . q[b]  with  q = hidden @ W.
This avoids the [B,T,H]x[H,H] matmul entirely and makes the problem
memory-bound: encoder_outputs (256 MB) is read exactly once from HBM.

Sharding: data-parallel over batch, 4 batches per core on 8 cores; W
replicated. Per core per batch: encoder tiles [128t, 1024h] stay resident
in SBUF (8 MB/batch, ~1.75 batches in flight), so the score pass (DVE fused
multiply+accumulate against broadcast q) and the context pass (PE matmuls
contracting over t) share one DMA read.
"""

import numpy as np

B, T, H = 32, 2048, 1024
NCORES = 8
BPC = B // NCORES  # batches per core
NT = T // 128      # 16 row-tiles of 128 per batch
HC = H // 128      # 8 column-chunks of 128

_CACHE = {}


def _split_multi_waits(nc, mybir):
    """Walrus in this env encodes at most 1 sync-wait per instruction for
    several encodings (Drain, LDWEIGHTS, ...). Split extra waits onto
    standalone EventSemaphore instrs placed just before, on the same engine
    (engines are sequential, so ordering is preserved)."""
    for fn in nc.m.functions:
        for blk in fn.blocks:
            new_insts = []
            for inst in blk.instructions:
                si = inst.sync_info
                if si is not None and len(si.on_wait) > 1:
                    waits = list(si.on_wait)
                    for j, w in enumerate(waits[:-1]):
                        new_insts.append(
                            mybir.InstEventSemaphore(
                                name=f"{inst.name}-wsplit{j}",
                                engine=inst.engine,
                                sync_info=mybir.SyncInfo(on_wait=[w], on_update=[]),
                            )
                        )
                    inst.sync_info = mybir.SyncInfo(
                        on_wait=[waits[-1]], on_update=list(si.on_update)
                    )
                new_insts.append(inst)
            blk.instructions = new_insts


def build_nc(split_waits=True):
    import concourse.bass as bass
    import concourse.tile as tile
    from concourse import mybir
    from concourse.masks import make_identity

    f32 = mybir.dt.float32
    f32r_ = mybir.dt.float32r
    nc = bass.Bass(trn_type="TRN2")

    hid = nc.dram_tensor("hidden", [BPC, H], f32, kind="ExternalInput")
    enc = nc.dram_tensor("enc", [BPC, T, H], f32r_, kind="ExternalInput")
    Wt = nc.dram_tensor("W", [H, H], f32, kind="ExternalInput")
    ctx_out = nc.dram_tensor("ctx", [BPC, H], f32, kind="ExternalOutput")
    attn_out = nc.dram_tensor("attn", [BPC, T], f32, kind="ExternalOutput")

    with tile.TileContext(nc) as tc:
        with (
            tc.tile_pool(name="singles", bufs=1) as singles,
            tc.tile_pool(name="qb", bufs=BPC) as qb_pool,
            tc.tile_pool(name="enc", bufs=7) as enc_pool,
            tc.tile_pool(name="wpool", bufs=2 * HC) as wpool,
            tc.tile_pool(name="scr", bufs=3) as scr_pool,
            tc.tile_pool(name="score", bufs=2) as score_pool,
            tc.tile_pool(name="sm", bufs=2) as sm_pool,
            tc.tile_pool(name="psum_big", bufs=2, space="PSUM") as psum_big,
            tc.tile_pool(name="psum_small", bufs=3, space="PSUM") as psum_small,
            tc.tile_pool(name="psum_warm", bufs=1, space="PSUM") as psum_warm,
        ):
            identity = singles.tile([128, 128], f32)
            make_identity(nc, identity)
            ones = singles.tile([128, 128], f32)
            nc.vector.memset(ones, 1.0)

            # hidden transposed with batch b at column 32*b: the lhsT
            # column index IS the PSUM output partition, so q rows land on
            # partitions {0,32,64,96} — all legal matmul rhs base partitions.
            # Columns in between are never read (their PSUM rows are unused).
            M97 = 32 * (BPC - 1) + 1
            hT = singles.tile([128, HC, M97], f32)
            nc.gpsimd.memset(hT, 0.0)
            # PE warm-up: ~5us of dummy matmuls so the HAM clock-gate opens
            # before the q chain starts (cold PE runs fp32 matmuls at half
            # speed; the gate needs ~3.4us of sustained activity).
            def pe_warm(n):
                for _ in range(n):
                    warm_ps = psum_warm.tile([128, 128], f32, tag="warm")
                    nc.tensor.matmul(
                        warm_ps, lhsT=ones, rhs=ones, start=True, stop=True
                    )

            pe_warm(14)

            hre = hid.rearrange("b (c p) -> p c b", p=128)  # [128, HC, BPC]
            hslc = hT[:, 0, 0:1]
            el = hslc.ap[-1][0]
            for b in range(BPC):
                dst = bass.AP(
                    tensor=hslc.tensor,
                    offset=hslc.offset + 32 * b * el,
                    ap=[list(hslc.ap[0]), [M97 * el, HC]],
                )
                nc.sync.dma_start(out=dst, in_=hre[:, :, b])

            # W loaded as 16 half-chunks so the q matmul chain pipelines
            # with W DMA arrival instead of waiting for all 4 MB.
            w_half = [[None] * 2 for _ in range(HC)]
            for c in range(HC):
                for hh in range(2):
                    wtile = wpool.tile([128, 512], f32, tag="w")
                    nc.sync.dma_start(
                        out=wtile,
                        in_=Wt[c * 128 : (c + 1) * 128, hh * 512 : hh * 512 + 512],
                    )
                    w_half[c][hh] = wtile

            # q = hidden @ W in one matmul chain; with the spread hT the
            # q rows land on PSUM partitions {0,32,64,96}
            q_ps = psum_big.tile([M97, H], f32, tag="big")
            for c in range(HC):
                for hh in range(2):
                    nc.tensor.matmul(
                        q_ps[:, hh * 512 : hh * 512 + 512],
                        lhsT=hT[:, c, :],
                        rhs=w_half[c][hh],
                        start=(c == 0),
                        stop=(c == HC - 1),
                    )
            q_sb = singles.tile([M97, H], f32)
            for b in range(BPC):
                nc.scalar.copy(
                    q_sb[32 * b : 32 * b + 1, :], q_ps[32 * b : 32 * b + 1, :]
                )

            # broadcast q[b] across 128 partitions via ones-matmul reading
            # directly from partition 32b (no DMA on the q critical path)
            q_bcast = []
            for b in range(BPC):
                qb = qb_pool.tile([128, H], f32, tag="qb")
                for h in range(0, H, 512):
                    ps = psum_small.tile([128, 512], f32, tag="small")
                    nc.tensor.matmul(
                        ps,
                        lhsT=ones[32 * b : 32 * b + 1, :],
                        rhs=q_sb[32 * b : 32 * b + 1, h : h + 512],
                        start=True, stop=True,
                        tile_position=(32 * b, 0),
                    )
                    nc.scalar.copy(qb[:, h : h + 512], ps)
                q_bcast.append(qb)

            f32r = mybir.dt.float32r
            Exp = mybir.ActivationFunctionType.Exp
            NCH = 4                    # row-tiles per DMA chunk (2 MB)
            HNT = NT // 2              # tiles per half-batch
            for b in range(BPC):
                # Half-batch "flash" structure: each half gets its own local
                # max, exp, running sum and PSUM context accumulation; the
                # two halves are combined exactly at the end. This lets the
                # context matmuls of the first half (and its enc slots)
                # retire while the second half is still streaming in.
                score = score_pool.tile([128, NT], f32, tag="score")
                wexp = score_pool.tile([128, NT], f32r_, tag="wexp")
                negm2 = sm_pool.tile([1, 2], f32, tag="negm2")
                s2 = sm_pool.tile([1, 2], f32, tag="s2")
                enc_tiles = []
                ctx_halves = []
                for half in range(2):
                    sl = slice(half * HNT, (half + 1) * HNT)
                    # ---- load enc chunks + fused mult+reduce -> scores
                    chunk_sizes = (
                        [4, 2, 2] if (b == BPC - 1 and half == 1) else [4, 4]
                    )
                    t0h = half * HNT
                    for csz in chunk_sizes:
                        ch = enc_pool.tile([128, NCH, H], f32r_, tag="enc")
                        nc.sync.dma_start(
                            out=ch[:, 0:csz, :],
                            in_=enc[
                                b, t0h * 128 : (t0h + csz) * 128, :
                            ].rearrange("(j p) h -> p j h", p=128),
                        )
                        for j in range(csz):
                            i = t0h + j
                            et = ch[:, j, :]
                            enc_tiles.append(et)
                            scr = scr_pool.tile([128, H], f32, tag="scr")
                            nc.vector.scalar_tensor_tensor(
                                out=scr,
                                in0=et.bitcast(f32),
                                scalar=1.0,
                                in1=q_bcast[b],
                                op0=mybir.AluOpType.mult,
                                op1=mybir.AluOpType.mult,
                                accum_out=score[:, i : i + 1],
                            )
                        t0h += csz

                    # ---- local softmax for this half (layout [128p, 8i])
                    m_p = sm_pool.tile([128, 1], f32, tag="mp")
                    nc.vector.tensor_reduce(
                        m_p, score[:, sl], axis=mybir.AxisListType.X,
                        op=mybir.AluOpType.max,
                    )
                    tp = psum_small.tile([1, 128], f32, tag="small")
                    nc.tensor.transpose(tp, m_p, identity)
                    nc.vector.tensor_reduce(
                        negm2[0:1, half : half + 1], tp,
                        axis=mybir.AxisListType.X,
                        op=mybir.AluOpType.max, negate=True,
                    )
                    nb_ps = psum_small.tile([128, 1], f32, tag="small")
                    nc.tensor.matmul(
                        nb_ps, lhsT=ones[0:1, :], rhs=negm2[0:1, half : half + 1],
                        start=True, stop=True,
                    )
                    negb = sm_pool.tile([128, 1], f32, tag="negb")
                    nc.scalar.copy(negb, nb_ps)
                    esum = sm_pool.tile([128, 1], f32, tag="esum")
                    nc.scalar.activation(
                        wexp[:, sl], score[:, sl], func=Exp,
                        bias=negb, accum_out=esum,
                    )
                    tp2 = psum_small.tile([1, 128], f32, tag="small")
                    nc.tensor.transpose(tp2, esum, identity)
                    nc.vector.tensor_reduce(
                        s2[0:1, half : half + 1], tp2,
                        axis=mybir.AxisListType.X, op=mybir.AluOpType.add,
                    )

                    # ---- context accumulation (float32r streams at
                    # 1 cyc/col; fp32 pays 4x; the truncation is harmless
                    # for a weighted average). Half B's matmuls are emitted
                    # AFTER the combine scalars below so the scheduler can
                    # interleave the combine/attn ops with the final burst.
                    if half == 0:
                        ctx_ps = psum_big.tile([1, H], f32, tag="big")
                        for i in range(HNT):
                            for h in (0, 512):
                                nc.tensor.matmul(
                                    ctx_ps[0:1, h : h + 512],
                                    lhsT=wexp[:, i : i + 1],
                                    rhs=enc_tiles[i][:, h : h + 512],
                                    start=(i == 0),
                                    stop=(i == HNT - 1),
                                )
                        ctx_halves.append(ctx_ps)

                # ---- combine halves exactly (no Ln — use exact
                # reciprocal so attention weights stay fp32-accurate):
                #   M = max(gA, gB); f_h = exp(g_h - M); Z = SA*fA + SB*fB
                #   attn = exp(score - M) / Z; ctx = (ctxA*fA + ctxB*fB)/Z
                negM = sm_pool.tile([1, 1], f32, tag="negM")
                nc.vector.tensor_reduce(
                    negM, negm2, axis=mybir.AxisListType.X, op=mybir.AluOpType.min
                )
                dAB = sm_pool.tile([1, 2], f32, tag="dAB")  # = M - g_h >= 0
                nc.vector.tensor_scalar(
                    dAB, negm2, negM[0:1, 0:1], None,
                    op0=mybir.AluOpType.subtract,
                )
                eAB = sm_pool.tile([1, 2], f32, tag="eAB")  # f_h = exp(g_h - M)
                nc.scalar.activation(eAB, dAB, func=Exp, scale=-1.0)
                zAB = sm_pool.tile([1, 2], f32, tag="zAB")
                nc.vector.tensor_mul(zAB, s2, eAB)
                Zt = sm_pool.tile([1, 1], f32, tag="Zt")
                nc.vector.tensor_reduce(
                    Zt, zAB, axis=mybir.AxisListType.X, op=mybir.AluOpType.add
                )
                rZ = sm_pool.tile([1, 1], f32, tag="rZ")
                nc.vector.reciprocal(rZ, Zt)
                fr = sm_pool.tile([1, 2], f32, tag="fr")  # f_h / Z
                nc.vector.tensor_scalar_mul(fr, eAB, rZ[0:1, 0:1])

                # scale half-A context now (only needs fr) so ACT runs it
                # during the half-B context burst below
                t1 = sm_pool.tile([1, H], f32, tag="t1")
                nc.scalar.mul(t1, ctx_halves[0], fr[0:1, 0:1])

                # normalized attention weights: exp(score - M) * (1/Z),
                # all in exact fp32 (independent of the f32r context path)
                nb2_ps = psum_small.tile([128, 1], f32, tag="small")
                nc.tensor.matmul(
                    nb2_ps, lhsT=ones[0:1, :], rhs=negM, start=True, stop=True
                )
                negbM = sm_pool.tile([128, 1], f32, tag="negbM")
                nc.vector.tensor_copy(negbM, nb2_ps)
                wg = score_pool.tile([128, NT], f32, tag="wg")
                nc.scalar.activation(wg, score, func=Exp, bias=negbM)
                rp_ps = psum_small.tile([128, 1], f32, tag="small")
                nc.tensor.matmul(rp_ps, lhsT=ones[0:1, :], rhs=rZ, start=True, stop=True)
                rp = sm_pool.tile([128, 1], f32, tag="rp")
                nc.vector.tensor_copy(rp, rp_ps)
                wn = score_pool.tile([128, NT], f32, tag="wn")
                nc.vector.tensor_scalar_mul(wn, wg, rp)

                # attn out: transpose [128,16] -> [16,128] so the DRAM write
                # is contiguous (t = i*128 + p)
                wt_ps = psum_small.tile([16, 128], f32, tag="small")
                nc.tensor.transpose(wt_ps, wn, identity)
                wn_t = sm_pool.tile([16, 128], f32, tag="wnt")
                nc.vector.tensor_copy(wn_t, wt_ps)
                nc.sync.dma_start(
                    out=attn_out[b].rearrange("(i p) -> i p", p=128), in_=wn_t
                )

                # half-B context burst (emitted after the combine/attn ops
                # so their engine-stream order lets them overlap this burst)
                ctxB_ps = psum_big.tile([1, H], f32, tag="big")
                for i in range(HNT, NT):
                    for h in (0, 512):
                        nc.tensor.matmul(
                            ctxB_ps[0:1, h : h + 512],
                            lhsT=wexp[:, i : i + 1],
                            rhs=enc_tiles[i][:, h : h + 512],
                            start=(i == HNT),
                            stop=(i == NT - 1),
                        )
                ctx_halves.append(ctxB_ps)

                # ctx = t1 + ctxB*(fB/Z): one fused op after the last matmul
                ctx_sb = sm_pool.tile([1, H], f32, tag="ctxsb")
                nc.vector.scalar_tensor_tensor(
                    out=ctx_sb,
                    in0=ctx_halves[1],
                    scalar=fr[0:1, 1:2],
                    in1=t1,
                    op0=mybir.AluOpType.mult,
                    op1=mybir.AluOpType.add,
                )
                nc.sync.dma_start(out=ctx_out[b : b + 1, :], in_=ctx_sb)

    if split_waits:
        _split_multi_waits(nc, mybir)
    return nc


def _get_runner():
    """Build (once) a cached jitted shard_map callable running the Bass NEFF
    on 8 cores. Mirrors concourse.bass2jax.run_bass_via_pjrt, but reusable
    across calls so the NEFF compiles exactly once per process."""
    if "runner" in _CACHE:
        return _CACHE["runner"]

    import jax
    from jax.experimental.shard_map import shard_map
    from jax.sharding import Mesh, PartitionSpec
    from concourse import mybir
    from concourse.bass2jax import (
        _bass_exec_p,
        install_neuronx_cc_hook,
        partition_id_tensor,
    )

    nc = build_nc()
    install_neuronx_cc_hook()

    partition_name = nc.partition_id_tensor.name if nc.partition_id_tensor else None
    in_names, out_names, out_avals, zero_outs = [], [], [], []
    for alloc in nc.m.functions[0].allocations:
        if not isinstance(alloc, mybir.MemoryLocationSet):
            continue
        name = alloc.memorylocations[0].name
        if alloc.kind == "ExternalInput":
            if name != partition_name:
                in_names.append(name)
        elif alloc.kind == "ExternalOutput":
            out_names.append(name)
            shape = tuple(alloc.tensor_shape)
            dtype = mybir.dt.np(alloc.dtype)
            out_avals.append(jax.core.ShapedArray(shape, dtype))
            zero_outs.append(np.zeros(shape, dtype))
    n_params = len(in_names)
    param_names = list(in_names)
    in_names = in_names + out_names
    if partition_name is not None:
        in_names.append(partition_name)
    donate = tuple(range(n_params, n_params + len(out_names)))

    def _body(*args):
        operands = list(args)
        if partition_name is not None:
            operands.append(partition_id_tensor())
        return tuple(
            _bass_exec_p.bind(
                *operands,
                out_avals=tuple(out_avals),
                in_names=tuple(in_names),
                out_names=tuple(out_names),
                lowering_input_output_aliases=(),
                sim_require_finite=True,
                sim_require_nnan=True,
                nc=nc,
            )
        )

    devices = jax.devices()[:NCORES]
    mesh = Mesh(np.asarray(devices), ("core",))
    in_specs = (PartitionSpec("core"),) * (n_params + len(out_names))
    out_specs = (PartitionSpec("core"),) * len(out_names)
    sharded = jax.jit(
        shard_map(
            _body, mesh=mesh, in_specs=in_specs, out_specs=out_specs,
            check_rep=False,
        ),
        donate_argnums=donate,
        keep_unused=True,
    )
    runner = {
        "fn": sharded,
        "param_names": param_names,
        "out_names": out_names,
        "out_avals": out_avals,
        "mesh": mesh,
    }
    _CACHE["runner"] = runner
    return runner


def _pack_inputs(hidden, encoder_outputs, W):
    """Concatenate per-core shards along axis 0 (shard_map splits axis 0)."""
    hidden = np.ascontiguousarray(hidden, dtype=np.float32)
    enc = np.ascontiguousarray(encoder_outputs, dtype=np.float32)
    W = np.ascontiguousarray(W, dtype=np.float32)
    full = {
        "hidden": hidden,                      # [32, H] -> 8 x [4, H]
        "enc": enc,                            # [32, T, H] -> 8 x [4, T, H]
        "W": np.concatenate([W] * NCORES, 0),  # replicated -> [8*H, H]
    }
    return full


def _fresh_zeros():
    r = _get_runner()
    return [
        np.zeros((NCORES * a.shape[0], *a.shape[1:]), a.dtype)
        for a in r["out_avals"]
    ]


def kernel(hidden, encoder_outputs, W):
    r = _get_runner()
    full = _pack_inputs(hidden, encoder_outputs, W)
    args = [full[name] for name in r["param_names"]]
    outs = r["fn"](*args, *_fresh_zeros())
    outd = {name: np.asarray(o) for name, o in zip(r["out_names"], outs)}
    return outd["ctx"], outd["attn"]
